# revision 46
# baseline (speedup 1.0000x reference)
"""Batched sparse-dense matmul (COO SpMM) on 8 Trainium2 NeuronCores.

Problem: y[b, r] = sum_k vals[k] * x[b, cols[k]] where rows[k] == r.
  x: [128, 16384] f32, vals/rows/cols: [524288], y: [128, 8192] f32.

Strategy: at 0.39% density with a full 128-wide batch, a dense matmul
y = x @ M^T beats per-nonzero gather formulations on this hardware (the
gather intermediate is NNZ*B elements ~ half the dense stream, and no
engine processes it faster than the HWDGE dense stream runs).  So:
  - Host: densify M^T into W [C, R] (a format conversion of the matrix,
    analogous to CSR/ELL packing), shard W's output columns across the
    8 cores (1024 rows each), and pre-tile both x^T and W for the SBUF
    partition layout.  W is cast to float8e3 (e3m4, 4 mantissa bits):
    the output error equals the W quantization error, ~1.34e-2 vs the
    2e-2 gate, and the W stream halves vs fp16 (16.8 MB/core), which
    flips the kernel from DMA-bound (~90 us stream floor) to PE-bound
    (~55 us of moving cycles; fp8 moving runs at bf16 speed without
    DoubleRow, and DoubleRow needs e4m3 whose 2.65e-2 error fails the
    gate).  x stays fp16 (adds ~3e-4, negligible); the PE multiplies
    mixed fp16 stationary x fp8 moving exactly into f32 PSUM.
  - Device (per core): keep x^T resident in SBUF as 128 [128c x 128b]
    chunks (the matmul's stationary operand); stream W from HBM in
    ~1MB tiles split byte-balanced across the two HWDGE rings
    (sync/scalar).  The stream is ordered r-major: PW-wide row passes
    over all 128 c-chunks, so each y slice accumulates in one PSUM
    bank, is copied out of PSUM, and its writeback DMA overlaps the
    next pass (a c-major order would serialize ALL of y's copy+DMA
    after the last W byte).  The y DMA issue is deferred behind two of
    the next pass's W tiles so it cannot head-of-line-block the
    in-order ring sequencer; the final pass tapers to 1-chunk tiles so
    the last accumulate chain starts as early as possible, and its
    PSUM copy + writeback are split across DVE+ACT and both rings to
    halve the tail-serial chain.
    With the fp8 W stream (21.5 MB/core total) the PE's 55 us of
    moving cycles is the critical path: pass 0 leads with small W
    tiles on alternating rings so the accumulate chain starts ~10.5 us
    in, and xt slices are issued with one-tile lookahead so they queue
    behind the W tiles the chain needs first.  Measures ~80-87 us
    end-to-end (vs ~112 us for the fp16 stream and 117-130 us for the
    original c-major fp16 baseline), run-to-run variance from HBM
    contention on the shared device.
  - Host: concatenate the per-core row slices.

Set DTYPE = "f32" for an exact (2e-5 absmax) variant at ~2x the time.
"""

import sys

sys.path.insert(0, "/opt/trn_rl_repo")

import numpy as np

import concourse.bacc as bacc
import concourse.mybir as mybir
import concourse.tile as tile
from concourse.bass_utils import run_bass_kernel_spmd

B = 128        # batch
R = 8192       # rows of sparse matrix / output features
C = 16384      # cols of sparse matrix / input features
NCORES = 8
RC = R // NCORES       # rows (output features) per core
NCH = C // 128         # contraction chunks of 128
PW = 512               # pass width (PSUM columns per pass)
NT = RC // PW          # passes per core

DTYPE = "f8"           # "f8" (W in fp8e3m4, ~1.3e-2 rel err), "f16"
                       # (~3e-4), or "f32" (exact)
import ml_dtypes

# W (moving operand) dtype / x^T (stationary) dtype per mode.  In "f8"
# mode only W is quantized to float8e3 (e3m4, 4 mantissa bits): the
# output error equals the W quantization error (~1.34e-2 for unit
# normals, measured), half the 2e-2 gate; x stays fp16 so it adds
# nothing.  The PE runs fp8 moving at bf16 speed (no DoubleRow), so
# this trades the DMA-bound 33.5 MB fp16 W stream for a PE-bound
# kernel at half the bytes.
_NP_W = {"f8": ml_dtypes.float8_e3m4, "f16": np.float16, "f32": np.float32}
_MY_W = {"f8": mybir.dt.float8e3, "f16": mybir.dt.float16, "f32": mybir.dt.float32}
_NP_X = {"f8": np.float16, "f16": np.float16, "f32": np.float32}
_MY_X = {"f8": mybir.dt.float16, "f16": mybir.dt.float16, "f32": mybir.dt.float32}


def _densify_tiled(vals, rows, cols):
    """w_t[p, ch, r] = sum of vals at (row=r, col=ch*128+p): dense M^T
    pre-tiled for the SBUF partition layout, [128, NCH, R] f32."""
    w_t = np.zeros((128, NCH, R), dtype=np.float32)
    np.add.at(w_t, (cols % 128, cols // 128, rows), vals)
    return w_t


def _pass_tiles(is_first, is_last, grp, taper):
    """Chunk tiling of one pass: (c0, csz) pairs covering NCH chunks.
    The first pass leads with one small tile so the accumulate chain
    (the PE critical path in fp8 mode) starts as soon as chunk 0
    lands — more than one small lead tile costs ~1us of serialized
    per-DMA ring overhead each and stalls the chain instead.  When the
    stream (not the PE) is the bottleneck (`taper`), the last pass
    steps down so the final chain finishes right after its last small
    tile lands; in fp8 mode the stream finishes far ahead of the PE
    and tapering is pure per-DMA overhead, so it is skipped."""
    tiles = []
    c0 = 0
    if is_first:
        for csz in (4, 8, 12):
            tiles.append((c0, csz))
            c0 += csz
    while NCH - c0 > (16 if taper and is_last else 0):
        csz = min(grp, NCH - c0)
        tiles.append((c0, csz))
        c0 += csz
    if taper and is_last:
        for csz in (8, 4, 1, 1, 1, 1):
            tiles.append((c0, csz))
            c0 += csz
    assert c0 == NCH
    return tiles


def _build_nc(dtype):
    wdt = _MY_W[dtype]
    xdt = _MY_X[dtype]
    wsz = mybir.dt.size(wdt)
    # keep 8 KB contiguous per partition per W tile (the measured
    # descriptor sweet spot; 16 KB tiles measured WORSE)
    grp = 8192 // (PW * wsz)
    nc = bacc.Bacc("TRN2", target_bir_lowering=False, debug=False)
    # x^T pre-tiled on host: xt[p, ch, b] = x[b, ch*128+p]
    xt_d = nc.dram_tensor("xt", [128, NCH * B], xdt, kind="ExternalInput")
    # W pre-tiled on host: w[p, t, ch, j] = W[ch*128+p, core_rows[t*PW+j]]
    w_d = nc.dram_tensor("w", [128, NT, NCH, PW], wdt, kind="ExternalInput")
    y_d = nc.dram_tensor("y", [128, RC], mybir.dt.float32, kind="ExternalOutput")

    with tile.TileContext(nc) as tc:
        with (
            tc.tile_pool(name="xsb", bufs=1) as xpool,
            tc.tile_pool(name="wsb", bufs=18 if dtype == "f8" else 10) as wpool,
            tc.tile_pool(name="ysb", bufs=1) as ypool,
            tc.tile_pool(name="ps", bufs=2, space="PSUM") as ppool,
        ):
            # greedy byte-balancing across the two HWDGE rings so both
            # finish together (the old fixed alternation left one ring
            # ~4MB behind, idling half the tail)
            ring_bytes = [0, 0]
            ring_eng = [nc.sync, nc.scalar]

            def ring(nbytes):
                i = 0 if ring_bytes[0] <= ring_bytes[1] else 1
                ring_bytes[i] += nbytes
                return ring_eng[i]

            x_t = xpool.tile([128, NCH, B], xdt)
            y_t = ypool.tile([128, RC], mybir.dt.float32)

            # x^T loads split and interleaved with pass 0's W stream so
            # the first matmuls start as soon as slice 0 lands; slice 0
            # is tiny since it gates the first matmul of the chain
            xbounds = [0, 2, 16, 32, 48, 64, 80, 96, 112, NCH]
            xt_issued = 0
            xesz = mybir.dt.size(xdt)

            def _load_xt_upto(ch_needed):
                nonlocal xt_issued
                while (
                    xt_issued < len(xbounds) - 1
                    and xbounds[xt_issued] <= ch_needed
                ):
                    lo, hi = xbounds[xt_issued], xbounds[xt_issued + 1]
                    ring((hi - lo) * B * 128 * xesz).dma_start(
                        out=x_t[:, lo:hi, :],
                        in_=xt_d[:, lo * B:hi * B],
                    )
                    xt_issued += 1

            # startup pinning: the sync ring's first packets land ~2.5us
            # before the scalar ring's (the ACT preamble is longer), so
            # put the xt head -- which gates the LDWEIGHTS of the first
            # chain chunks -- on sync, and the first W tiles on scalar;
            # greedy balancing resumes after
            for s in (0, 1):
                lo, hi = xbounds[s], xbounds[s + 1]
                ring_bytes[0] += (hi - lo) * B * 128 * xesz
                ring_eng[0].dma_start(
                    out=x_t[:, lo:hi, :], in_=xt_d[:, lo * B:hi * B]
                )
            xt_issued = 2
            # the chain's first W tile follows the xt head on sync (its
            # packets start earliest); the next lead tiles go on scalar

            # pass-t y writeback is deferred until a couple of pass-t+1
            # W tiles are issued: the ring sequencers are in-order, so an
            # immediately-issued y DMA (waiting on the PSUM copy) would
            # head-of-line-block the next pass's stream
            pending_y = None

            def _flush_y(eng=None):
                nonlocal pending_y
                if pending_y is None:
                    return
                sl = pending_y
                pending_y = None
                (eng or ring(PW * 128 * 4)).dma_start(
                    out=y_d[:, sl], in_=y_t[:, sl]
                )

            for t in range(NT):
                psum = ppool.tile(
                    [128, PW], mybir.dt.float32, name=f"psum{t}", tag=f"psum{t}"
                )
                for k, (c0, csz) in enumerate(
                    _pass_tiles(t == 0, t == NT - 1, grp, dtype != "f8")
                ):
                    w_t = wpool.tile([128, grp, PW], wdt)
                    wbytes = csz * PW * 128 * wsz
                    if t == 0 and k < 3:
                        i = 0 if k == 0 else 1
                        ring_bytes[i] += wbytes
                        weng = ring_eng[i]
                    else:
                        weng = ring(wbytes)
                    weng.dma_start(
                        out=w_t[:, :csz, :], in_=w_d[:, t, c0:c0 + csz, :]
                    )
                    if t == 0:
                        # one-tile + sem-prop lookahead: xt slices queue
                        # BEHIND the W tiles the PE chain needs first
                        # (xt stealing ring bandwidth ahead of W starves
                        # the PE), but far enough ahead that their ~900ns
                        # completion-semaphore latency is hidden
                        _load_xt_upto(min(c0 + csz + 8, NCH - 1))
                    if k == 2:
                        _flush_y()
                    for i in range(csz):
                        ch = c0 + i
                        nc.tensor.matmul(
                            psum[:],
                            x_t[:, ch, :],
                            w_t[:, i, :],
                            start=(ch == 0),
                            stop=(ch == NCH - 1),
                        )
                if t < NT - 1:
                    nc.vector.tensor_copy(
                        out=y_t[:, t * PW:(t + 1) * PW], in_=psum[:]
                    )
                    pending_y = slice(t * PW, (t + 1) * PW)
                else:
                    # last pass is tail-serial: split the PSUM copy across
                    # DVE + ACT and the writeback across both rings so the
                    # final chain is half as long
                    h = PW // 2
                    lo = slice(t * PW, t * PW + h)
                    hi = slice(t * PW + h, (t + 1) * PW)
                    nc.vector.tensor_copy(out=y_t[:, lo], in_=psum[:, :h])
                    nc.scalar.copy(out=y_t[:, hi], in_=psum[:, h:])
                    nc.sync.dma_start(out=y_d[:, lo], in_=y_t[:, lo])
                    nc.scalar.dma_start(out=y_d[:, hi], in_=y_t[:, hi])
    nc.compile()
    return nc


_CACHE = {}
_TRACE = False  # set by bench harness to capture an NTFF profile


def _get_nc(dtype):
    if dtype not in _CACHE:
        _CACHE[dtype] = _build_nc(dtype)
    return _CACHE[dtype]


def kernel(x_batched, M_vals, M_row_idx, M_col_idx, _want_results=False, **_):
    x = np.asarray(x_batched, dtype=np.float32)
    vals = np.asarray(M_vals, dtype=np.float32)
    rows = np.asarray(M_row_idx, dtype=np.int64)
    cols = np.asarray(M_col_idx, dtype=np.int64)

    w_t = _densify_tiled(vals, rows, cols).astype(_NP_W[DTYPE])  # [128, NCH, R]
    xt = np.ascontiguousarray(
        x.T.reshape(NCH, 128, B).transpose(1, 0, 2).reshape(128, NCH * B)
    ).astype(_NP_X[DTYPE])

    nc = _get_nc(DTYPE)
    in_maps = []
    for m in range(NCORES):
        # [128, NCH, RC] -> [128, NT, NCH, PW] (r-major pass layout)
        shard = w_t[:, :, m * RC:(m + 1) * RC]
        shard = np.ascontiguousarray(
            shard.reshape(128, NCH, NT, PW).transpose(0, 2, 1, 3)
        )
        in_maps.append({"xt": xt, "w": shard})
    try:
        res = run_bass_kernel_spmd(
            nc, in_maps, core_ids=list(range(NCORES)), trace=_TRACE
        )
    except Exception:
        # transient NRT/device wedges have been observed to clear on retry
        res = run_bass_kernel_spmd(
            nc, in_maps, core_ids=list(range(NCORES)), trace=_TRACE
        )

    y = np.empty((B, R), dtype=np.float32)
    for m in range(NCORES):
        y[:, m * RC:(m + 1) * RC] = res.results[m]["y"]
    if _want_results:
        return y, res
    return y


# revision 48
# speedup vs baseline: 1.1084x; 1.1084x over previous
"""Batched sparse-dense matmul (COO SpMM) on 8 Trainium2 NeuronCores.

Problem: y[b, r] = sum_k vals[k] * x[b, cols[k]] where rows[k] == r.
  x: [128, 16384] f32, vals/rows/cols: [524288], y: [128, 8192] f32.

Strategy: at 0.39% density with a full 128-wide batch, a dense matmul
y = x @ M^T beats per-nonzero gather formulations on this hardware (the
gather intermediate is NNZ*B elements ~ half the dense stream, and no
engine processes it faster than the HWDGE dense stream runs).  So:
  - Host: densify M^T into W [C, R] (a format conversion of the matrix,
    analogous to CSR/ELL packing), shard W's output columns across the
    8 cores (1024 rows each), and pre-tile both x^T and W for the SBUF
    partition layout.  W is cast to float8e3 (e3m4, 4 mantissa bits):
    the output error equals the W quantization error, ~1.34e-2 vs the
    2e-2 gate, and the W stream halves vs fp16 (16.8 MB/core), which
    flips the kernel from DMA-bound (~90 us stream floor) to PE-bound
    (~55 us of moving cycles; fp8 moving runs at bf16 speed without
    DoubleRow, and DoubleRow needs e4m3 whose 2.65e-2 error fails the
    gate).  x stays fp16 (adds ~3e-4, negligible); the PE multiplies
    mixed fp16 stationary x fp8 moving exactly into f32 PSUM.
  - Device (per core): keep x^T resident in SBUF as 128 [128c x 128b]
    chunks (the matmul's stationary operand); stream W from HBM in
    ~1MB tiles split byte-balanced across the two HWDGE rings
    (sync/scalar).  The stream is ordered r-major: PW-wide row passes
    over all 128 c-chunks, so each y slice accumulates in one PSUM
    bank, is copied out of PSUM, and its writeback DMA overlaps the
    next pass (a c-major order would serialize ALL of y's copy+DMA
    after the last W byte).  The y DMA issue is deferred behind two of
    the next pass's W tiles so it cannot head-of-line-block the
    in-order ring sequencer; the final pass tapers to 1-chunk tiles so
    the last accumulate chain starts as early as possible, and its
    PSUM copy + writeback are split across DVE+ACT and both rings to
    halve the tail-serial chain.
    With the fp8 W stream (21.5 MB/core total) the PE's 55 us of
    moving cycles is the critical path: pass 0 leads with small W
    tiles on alternating rings so the accumulate chain starts ~10.5 us
    in, and xt slices are issued with one-tile lookahead so they queue
    behind the W tiles the chain needs first.  Measures ~80-87 us
    end-to-end (vs ~112 us for the fp16 stream and 117-130 us for the
    original c-major fp16 baseline), run-to-run variance from HBM
    contention on the shared device.
  - Host: concatenate the per-core row slices.

Set DTYPE = "f32" for an exact (2e-5 absmax) variant at ~2x the time.
"""

import sys

sys.path.insert(0, "/opt/trn_rl_repo")

import numpy as np

import concourse.bacc as bacc
import concourse.mybir as mybir
import concourse.tile as tile
from concourse.bass_utils import run_bass_kernel_spmd

B = 128        # batch
R = 8192       # rows of sparse matrix / output features
C = 16384      # cols of sparse matrix / input features
NCORES = 8
RC = R // NCORES       # rows (output features) per core
NCH = C // 128         # contraction chunks of 128
PW = 512               # pass width (PSUM columns per pass)
NT = RC // PW          # passes per core

DTYPE = "f8"           # "f8" (W in fp8e3m4, ~1.3e-2 rel err), "f16"
                       # (~3e-4), or "f32" (exact)
import ml_dtypes

# W (moving operand) dtype / x^T (stationary) dtype per mode.  In "f8"
# mode only W is quantized to float8e3 (e3m4, 4 mantissa bits): the
# output error equals the W quantization error (~1.34e-2 for unit
# normals, measured), half the 2e-2 gate; x stays fp16 so it adds
# nothing.  The PE runs fp8 moving at bf16 speed (no DoubleRow), so
# this trades the DMA-bound 33.5 MB fp16 W stream for a PE-bound
# kernel at half the bytes.
_NP_W = {"f8": ml_dtypes.float8_e3m4, "f16": np.float16, "f32": np.float32}
_MY_W = {"f8": mybir.dt.float8e3, "f16": mybir.dt.float16, "f32": mybir.dt.float32}
_NP_X = {"f8": np.float16, "f16": np.float16, "f32": np.float32}
_MY_X = {"f8": mybir.dt.float16, "f16": mybir.dt.float16, "f32": mybir.dt.float32}


def _densify_tiled(vals, rows, cols):
    """w_t[p, ch, r] = sum of vals at (row=r, col=ch*128+p): dense M^T
    pre-tiled for the SBUF partition layout, [128, NCH, R] f32."""
    w_t = np.zeros((128, NCH, R), dtype=np.float32)
    np.add.at(w_t, (cols % 128, cols // 128, rows), vals)
    return w_t


def _pass_tiles(is_first, is_last, grp, taper):
    """Chunk tiling of one pass: (c0, csz) pairs covering NCH chunks.
    The first pass leads with one small tile so the accumulate chain
    (the PE critical path in fp8 mode) starts as soon as chunk 0
    lands — more than one small lead tile costs ~1us of serialized
    per-DMA ring overhead each and stalls the chain instead.  When the
    stream (not the PE) is the bottleneck (`taper`), the last pass
    steps down so the final chain finishes right after its last small
    tile lands; in fp8 mode the stream finishes far ahead of the PE
    and tapering is pure per-DMA overhead, so it is skipped."""
    tiles = []
    c0 = 0
    if is_first:
        for csz in (4, 8, 12):
            tiles.append((c0, csz))
            c0 += csz
    while NCH - c0 > (16 if taper and is_last else 0):
        csz = min(grp, NCH - c0)
        tiles.append((c0, csz))
        c0 += csz
    if taper and is_last:
        for csz in (8, 4, 1, 1, 1, 1):
            tiles.append((c0, csz))
            c0 += csz
    assert c0 == NCH
    return tiles


def _build_nc(dtype):
    wdt = _MY_W[dtype]
    xdt = _MY_X[dtype]
    wsz = mybir.dt.size(wdt)
    # keep 8 KB contiguous per partition per W tile (the measured
    # descriptor sweet spot; 16 KB tiles measured WORSE)
    grp = 8192 // (PW * wsz)
    nc = bacc.Bacc("TRN2", target_bir_lowering=False, debug=False)
    # x^T pre-tiled on host: xt[p, ch, b] = x[b, ch*128+p]
    xt_d = nc.dram_tensor("xt", [128, NCH * B], xdt, kind="ExternalInput")
    # W pre-tiled on host: w[p, t, ch, j] = W[ch*128+p, core_rows[t*PW+j]]
    w_d = nc.dram_tensor("w", [128, NT, NCH, PW], wdt, kind="ExternalInput")
    y_d = nc.dram_tensor("y", [128, RC], mybir.dt.float32, kind="ExternalOutput")

    with tile.TileContext(nc) as tc:
        with (
            tc.tile_pool(name="xsb", bufs=1) as xpool,
            tc.tile_pool(name="wsb", bufs=18 if dtype == "f8" else 10) as wpool,
            tc.tile_pool(name="ysb", bufs=1) as ypool,
            tc.tile_pool(name="ps", bufs=2, space="PSUM") as ppool,
        ):
            # greedy byte-balancing across the two HWDGE rings so both
            # finish together (the old fixed alternation left one ring
            # ~4MB behind, idling half the tail)
            ring_bytes = [0, 0]
            ring_eng = [nc.sync, nc.scalar]

            def ring(nbytes):
                i = 0 if ring_bytes[0] <= ring_bytes[1] else 1
                ring_bytes[i] += nbytes
                return ring_eng[i]

            x_t = xpool.tile([128, NCH, B], xdt)
            y_t = ypool.tile([128, RC], mybir.dt.float32)

            # x^T loads split and interleaved with pass 0's W stream so
            # the first matmuls start as soon as slice 0 lands; slice 0
            # is tiny since it gates the first matmul of the chain
            xbounds = [0, 2, 16, 32, 48, 64, 80, 96, 112, NCH]
            xt_issued = 0
            xesz = mybir.dt.size(xdt)

            def _load_xt_upto(ch_needed):
                nonlocal xt_issued
                while (
                    xt_issued < len(xbounds) - 1
                    and xbounds[xt_issued] <= ch_needed
                ):
                    lo, hi = xbounds[xt_issued], xbounds[xt_issued + 1]
                    ring((hi - lo) * B * 128 * xesz).dma_start(
                        out=x_t[:, lo:hi, :],
                        in_=xt_d[:, lo * B:hi * B],
                    )
                    xt_issued += 1

            # startup pinning: the sync ring's first packets land ~2.5us
            # before the scalar ring's (the ACT preamble is longer), so
            # put the xt head -- which gates the LDWEIGHTS of the first
            # chain chunks -- on sync, and the first W tiles on scalar;
            # greedy balancing resumes after
            # the chain-critical head transfers go out on the gpsimd
            # SWDGE ring: its engine preamble ends ~3us in (vs ~6.5us
            # for sync / ~9us for scalar before HWDGE packets flow), so
            # the first accumulate chunks are fed earliest
            for s in (0, 1):
                lo, hi = xbounds[s], xbounds[s + 1]
                nc.gpsimd.dma_start(
                    out=x_t[:, lo:hi, :], in_=xt_d[:, lo * B:hi * B]
                )
            xt_issued = 2

            # pass-t y writeback is deferred until a couple of pass-t+1
            # W tiles are issued: the ring sequencers are in-order, so an
            # immediately-issued y DMA (waiting on the PSUM copy) would
            # head-of-line-block the next pass's stream
            pending_y = None

            def _flush_y(eng=None):
                nonlocal pending_y
                if pending_y is None:
                    return
                sl = pending_y
                pending_y = None
                (eng or ring(PW * 128 * 4)).dma_start(
                    out=y_d[:, sl], in_=y_t[:, sl]
                )

            for t in range(NT):
                psum = ppool.tile(
                    [128, PW], mybir.dt.float32, name=f"psum{t}", tag=f"psum{t}"
                )
                for k, (c0, csz) in enumerate(
                    _pass_tiles(t == 0, t == NT - 1, grp, dtype != "f8")
                ):
                    w_t = wpool.tile([128, grp, PW], wdt)
                    wbytes = csz * PW * 128 * wsz
                    if t == 0 and k < 2:
                        weng = nc.gpsimd
                    elif t == 0 and k == 2:
                        ring_bytes[1] += wbytes
                        weng = ring_eng[1]
                    else:
                        weng = ring(wbytes)
                    weng.dma_start(
                        out=w_t[:, :csz, :], in_=w_d[:, t, c0:c0 + csz, :]
                    )
                    if t == 0:
                        # one-tile + sem-prop lookahead: xt slices queue
                        # BEHIND the W tiles the PE chain needs first
                        # (xt stealing ring bandwidth ahead of W starves
                        # the PE), but far enough ahead that their ~900ns
                        # completion-semaphore latency is hidden
                        _load_xt_upto(min(c0 + csz + 8, NCH - 1))
                    if k == 2:
                        _flush_y()
                    for i in range(csz):
                        ch = c0 + i
                        nc.tensor.matmul(
                            psum[:],
                            x_t[:, ch, :],
                            w_t[:, i, :],
                            start=(ch == 0),
                            stop=(ch == NCH - 1),
                        )
                if t < NT - 1:
                    nc.vector.tensor_copy(
                        out=y_t[:, t * PW:(t + 1) * PW], in_=psum[:]
                    )
                    pending_y = slice(t * PW, (t + 1) * PW)
                else:
                    # last pass is tail-serial: split the PSUM copy across
                    # DVE + ACT and the writeback across both rings so the
                    # final chain is half as long
                    h = PW // 2
                    lo = slice(t * PW, t * PW + h)
                    hi = slice(t * PW + h, (t + 1) * PW)
                    nc.vector.tensor_copy(out=y_t[:, lo], in_=psum[:, :h])
                    nc.scalar.copy(out=y_t[:, hi], in_=psum[:, h:])
                    nc.sync.dma_start(out=y_d[:, lo], in_=y_t[:, lo])
                    nc.scalar.dma_start(out=y_d[:, hi], in_=y_t[:, hi])
    nc.compile()
    return nc


_CACHE = {}
_TRACE = False  # set by bench harness to capture an NTFF profile


def _get_nc(dtype):
    if dtype not in _CACHE:
        _CACHE[dtype] = _build_nc(dtype)
    return _CACHE[dtype]


def kernel(x_batched, M_vals, M_row_idx, M_col_idx, _want_results=False, **_):
    x = np.asarray(x_batched, dtype=np.float32)
    vals = np.asarray(M_vals, dtype=np.float32)
    rows = np.asarray(M_row_idx, dtype=np.int64)
    cols = np.asarray(M_col_idx, dtype=np.int64)

    w_t = _densify_tiled(vals, rows, cols).astype(_NP_W[DTYPE])  # [128, NCH, R]
    xt = np.ascontiguousarray(
        x.T.reshape(NCH, 128, B).transpose(1, 0, 2).reshape(128, NCH * B)
    ).astype(_NP_X[DTYPE])

    nc = _get_nc(DTYPE)
    in_maps = []
    for m in range(NCORES):
        # [128, NCH, RC] -> [128, NT, NCH, PW] (r-major pass layout)
        shard = w_t[:, :, m * RC:(m + 1) * RC]
        shard = np.ascontiguousarray(
            shard.reshape(128, NCH, NT, PW).transpose(0, 2, 1, 3)
        )
        in_maps.append({"xt": xt, "w": shard})
    try:
        res = run_bass_kernel_spmd(
            nc, in_maps, core_ids=list(range(NCORES)), trace=_TRACE
        )
    except Exception:
        # transient NRT/device wedges have been observed to clear on retry
        res = run_bass_kernel_spmd(
            nc, in_maps, core_ids=list(range(NCORES)), trace=_TRACE
        )

    y = np.empty((B, R), dtype=np.float32)
    for m in range(NCORES):
        y[:, m * RC:(m + 1) * RC] = res.results[m]["y"]
    if _want_results:
        return y, res
    return y


# revision 50
# speedup vs baseline: 1.1760x; 1.0610x over previous
"""Batched sparse-dense matmul (COO SpMM) on 8 Trainium2 NeuronCores.

Problem: y[b, r] = sum_k vals[k] * x[b, cols[k]] where rows[k] == r.
  x: [128, 16384] f32, vals/rows/cols: [524288], y: [128, 8192] f32.

Strategy: at 0.39% density with a full 128-wide batch, a dense matmul
y = x @ M^T beats per-nonzero gather formulations on this hardware (the
gather intermediate is NNZ*B elements ~ half the dense stream, and no
engine processes it faster than the HWDGE dense stream runs).  So:
  - Host: densify M^T into W [C, R] (a format conversion of the matrix,
    analogous to CSR/ELL packing), shard W's output columns across the
    8 cores (1024 rows each), and pre-tile both x^T and W for the SBUF
    partition layout.  W is cast to float8e3 (e3m4, 4 mantissa bits):
    the output error equals the W quantization error, ~1.34e-2 vs the
    2e-2 gate, and the W stream halves vs fp16 (16.8 MB/core), which
    flips the kernel from DMA-bound (~90 us stream floor) to PE-bound
    (~55 us of moving cycles; fp8 moving runs at bf16 speed without
    DoubleRow, and DoubleRow needs e4m3 whose 2.65e-2 error fails the
    gate).  x stays fp16 (adds ~3e-4, negligible); the PE multiplies
    mixed fp16 stationary x fp8 moving exactly into f32 PSUM.
  - Device (per core): keep x^T resident in SBUF as 128 [128c x 128b]
    chunks (the matmul's stationary operand); stream W from HBM in
    ~1MB tiles split byte-balanced across the two HWDGE rings
    (sync/scalar).  The stream is ordered r-major: PW-wide row passes
    over all 128 c-chunks, so each y slice accumulates in one PSUM
    bank, is copied out of PSUM, and its writeback DMA overlaps the
    next pass (a c-major order would serialize ALL of y's copy+DMA
    after the last W byte).  The y DMA issue is deferred behind two of
    the next pass's W tiles so it cannot head-of-line-block the
    in-order ring sequencer; the final pass tapers to 1-chunk tiles so
    the last accumulate chain starts as early as possible, and its
    PSUM copy + writeback are split across DVE+ACT and both rings to
    halve the tail-serial chain.
    With the fp8 W stream (21.5 MB/core total) the PE's 55 us of
    moving cycles is the critical path: pass 0 leads with small W
    tiles on alternating rings so the accumulate chain starts ~10.5 us
    in, and xt slices are issued with one-tile lookahead so they queue
    behind the W tiles the chain needs first.  Measures ~80-87 us
    end-to-end (vs ~112 us for the fp16 stream and 117-130 us for the
    original c-major fp16 baseline), run-to-run variance from HBM
    contention on the shared device.
  - Host: concatenate the per-core row slices.

Set DTYPE = "f32" for an exact (2e-5 absmax) variant at ~2x the time.
"""

import sys

sys.path.insert(0, "/opt/trn_rl_repo")

import numpy as np

import concourse.bacc as bacc
import concourse.mybir as mybir
import concourse.tile as tile
from concourse.bass_utils import run_bass_kernel_spmd

B = 128        # batch
R = 8192       # rows of sparse matrix / output features
C = 16384      # cols of sparse matrix / input features
NCORES = 8
RC = R // NCORES       # rows (output features) per core
NCH = C // 128         # contraction chunks of 128
PW = 512               # pass width (PSUM columns per pass)
NT = RC // PW          # passes per core

DTYPE = "f8"           # "f8" (W in fp8e3m4, ~1.3e-2 rel err), "f16"
                       # (~3e-4), or "f32" (exact)
import ml_dtypes

# W (moving operand) dtype / x^T (stationary) dtype per mode.  In "f8"
# mode only W is quantized to float8e3 (e3m4, 4 mantissa bits): the
# output error equals the W quantization error (~1.34e-2 for unit
# normals, measured), half the 2e-2 gate; x stays fp16 so it adds
# nothing.  The PE runs fp8 moving at bf16 speed (no DoubleRow), so
# this trades the DMA-bound 33.5 MB fp16 W stream for a PE-bound
# kernel at half the bytes.
_NP_W = {"f8": ml_dtypes.float8_e3m4, "f16": np.float16, "f32": np.float32}
_MY_W = {"f8": mybir.dt.float8e3, "f16": mybir.dt.float16, "f32": mybir.dt.float32}
_NP_X = {"f8": np.float16, "f16": np.float16, "f32": np.float32}
_MY_X = {"f8": mybir.dt.float16, "f16": mybir.dt.float16, "f32": mybir.dt.float32}


def _densify_tiled(vals, rows, cols):
    """w_t[p, ch, r] = sum of vals at (row=r, col=ch*128+p): dense M^T
    pre-tiled for the SBUF partition layout, [128, NCH, R] f32."""
    w_t = np.zeros((128, NCH, R), dtype=np.float32)
    np.add.at(w_t, (cols % 128, cols // 128, rows), vals)
    return w_t


def _pass_tiles(is_first, is_last, grp, taper):
    """Chunk tiling of one pass: (c0, csz) pairs covering NCH chunks.
    The first pass leads with one small tile so the accumulate chain
    (the PE critical path in fp8 mode) starts as soon as chunk 0
    lands — more than one small lead tile costs ~1us of serialized
    per-DMA ring overhead each and stalls the chain instead.  When the
    stream (not the PE) is the bottleneck (`taper`), the last pass
    steps down so the final chain finishes right after its last small
    tile lands; in fp8 mode the stream finishes far ahead of the PE
    and tapering is pure per-DMA overhead, so it is skipped."""
    tiles = []
    c0 = 0
    if is_first:
        for csz in (4, 8, 12):
            tiles.append((c0, csz))
            c0 += csz
    while NCH - c0 > (16 if taper and is_last else 0):
        csz = min(grp, NCH - c0)
        tiles.append((c0, csz))
        c0 += csz
    if taper and is_last:
        for csz in (8, 4, 1, 1, 1, 1):
            tiles.append((c0, csz))
            c0 += csz
    assert c0 == NCH
    return tiles


def _build_nc(dtype):
    wdt = _MY_W[dtype]
    xdt = _MY_X[dtype]
    wsz = mybir.dt.size(wdt)
    # keep 8 KB contiguous per partition per W tile (the measured
    # descriptor sweet spot; 16 KB tiles measured WORSE)
    grp = 8192 // (PW * wsz)
    nc = bacc.Bacc("TRN2", target_bir_lowering=False, debug=False)
    # x^T pre-tiled on host: xt[p, ch, b] = x[b, ch*128+p]
    xt_d = nc.dram_tensor("xt", [128, NCH * B], xdt, kind="ExternalInput")
    # W pre-tiled on host: w[p, t, ch, j] = W[ch*128+p, core_rows[t*PW+j]]
    w_d = nc.dram_tensor("w", [128, NT, NCH, PW], wdt, kind="ExternalInput")
    y_d = nc.dram_tensor("y", [128, RC], mybir.dt.float32, kind="ExternalOutput")

    with tile.TileContext(nc) as tc:
        with (
            tc.tile_pool(name="xsb", bufs=1) as xpool,
            tc.tile_pool(name="wsb", bufs=18 if dtype == "f8" else 10) as wpool,
            tc.tile_pool(name="ysb", bufs=1) as ypool,
            tc.tile_pool(name="ps", bufs=2, space="PSUM") as ppool,
        ):
            # greedy byte-balancing across the two HWDGE rings so both
            # finish together (the old fixed alternation left one ring
            # ~4MB behind, idling half the tail)
            ring_bytes = [0, 0]
            ring_eng = [nc.sync, nc.scalar]

            def ring(nbytes):
                i = 0 if ring_bytes[0] <= ring_bytes[1] else 1
                ring_bytes[i] += nbytes
                return ring_eng[i]

            x_t = xpool.tile([128, NCH, B], xdt)
            y_t = ypool.tile([128, RC], mybir.dt.float32)

            # x^T loads split and interleaved with pass 0's W stream so
            # the first matmuls start as soon as slice 0 lands; slice 0
            # is tiny since it gates the first matmul of the chain
            xbounds = [0, 2, 16, 32, 48, 64, 80, 96, 112, NCH]
            xt_issued = 0
            xesz = mybir.dt.size(xdt)

            def _load_xt_upto(ch_needed):
                nonlocal xt_issued
                while (
                    xt_issued < len(xbounds) - 1
                    and xbounds[xt_issued] <= ch_needed
                ):
                    lo, hi = xbounds[xt_issued], xbounds[xt_issued + 1]
                    ring((hi - lo) * B * 128 * xesz).dma_start(
                        out=x_t[:, lo:hi, :],
                        in_=xt_d[:, lo * B:hi * B],
                    )
                    xt_issued += 1

            # startup pinning: the sync ring's first packets land ~2.5us
            # before the scalar ring's (the ACT preamble is longer), so
            # put the xt head -- which gates the LDWEIGHTS of the first
            # chain chunks -- on sync, and the first W tiles on scalar;
            # greedy balancing resumes after
            # startup pinning: the sync ring's packets start earliest
            # (~8.7us vs ~10.4us scalar, ~11.7us gpsimd/SWDGE -- the Q7
            # path pays a first-use IRAM load), so the xt head -- which
            # gates the LDWEIGHTS of the first chain chunks -- and the
            # first W tile go on sync; the next lead tiles on scalar
            for s in (0, 1):
                lo, hi = xbounds[s], xbounds[s + 1]
                ring_bytes[0] += (hi - lo) * B * 128 * xesz
                ring_eng[0].dma_start(
                    out=x_t[:, lo:hi, :], in_=xt_d[:, lo * B:hi * B]
                )
            xt_issued = 2

            # pass-t y writeback is deferred until a couple of pass-t+1
            # W tiles are issued: the ring sequencers are in-order, so an
            # immediately-issued y DMA (waiting on the PSUM copy) would
            # head-of-line-block the next pass's stream
            pending_y = None

            def _flush_y(eng=None):
                nonlocal pending_y
                if pending_y is None:
                    return
                sl = pending_y
                pending_y = None
                (eng or ring(PW * 128 * 4)).dma_start(
                    out=y_d[:, sl], in_=y_t[:, sl]
                )

            for t in range(NT):
                psum = ppool.tile(
                    [128, PW], mybir.dt.float32, name=f"psum{t}", tag=f"psum{t}"
                )
                for k, (c0, csz) in enumerate(
                    _pass_tiles(t == 0, t == NT - 1, grp, dtype != "f8")
                ):
                    w_t = wpool.tile([128, grp, PW], wdt)
                    wbytes = csz * PW * 128 * wsz
                    if t == 0 and k < 3:
                        i = 0 if k == 0 else 1
                        ring_bytes[i] += wbytes
                        weng = ring_eng[i]
                    else:
                        weng = ring(wbytes)
                    weng.dma_start(
                        out=w_t[:, :csz, :], in_=w_d[:, t, c0:c0 + csz, :]
                    )
                    if t == 0:
                        # one-tile + sem-prop lookahead: xt slices queue
                        # BEHIND the W tiles the PE chain needs first
                        # (xt stealing ring bandwidth ahead of W starves
                        # the PE), but far enough ahead that their ~900ns
                        # completion-semaphore latency is hidden
                        _load_xt_upto(min(c0 + csz + 8, NCH - 1))
                    if k == 2:
                        _flush_y()
                    for i in range(csz):
                        ch = c0 + i
                        nc.tensor.matmul(
                            psum[:],
                            x_t[:, ch, :],
                            w_t[:, i, :],
                            start=(ch == 0),
                            stop=(ch == NCH - 1),
                        )
                if t < NT - 1:
                    nc.vector.tensor_copy(
                        out=y_t[:, t * PW:(t + 1) * PW], in_=psum[:]
                    )
                    pending_y = slice(t * PW, (t + 1) * PW)
                else:
                    # last pass is tail-serial: split the PSUM copy across
                    # DVE + ACT and the writeback across both rings so the
                    # final chain is half as long
                    h = PW // 2
                    lo = slice(t * PW, t * PW + h)
                    hi = slice(t * PW + h, (t + 1) * PW)
                    nc.vector.tensor_copy(out=y_t[:, lo], in_=psum[:, :h])
                    nc.scalar.copy(out=y_t[:, hi], in_=psum[:, h:])
                    nc.sync.dma_start(out=y_d[:, lo], in_=y_t[:, lo])
                    nc.scalar.dma_start(out=y_d[:, hi], in_=y_t[:, hi])
    nc.compile()
    return nc


_CACHE = {}
_TRACE = False  # set by bench harness to capture an NTFF profile


def _get_nc(dtype):
    if dtype not in _CACHE:
        _CACHE[dtype] = _build_nc(dtype)
    return _CACHE[dtype]


def kernel(x_batched, M_vals, M_row_idx, M_col_idx, _want_results=False, **_):
    x = np.asarray(x_batched, dtype=np.float32)
    vals = np.asarray(M_vals, dtype=np.float32)
    rows = np.asarray(M_row_idx, dtype=np.int64)
    cols = np.asarray(M_col_idx, dtype=np.int64)

    w_t = _densify_tiled(vals, rows, cols).astype(_NP_W[DTYPE])  # [128, NCH, R]
    xt = np.ascontiguousarray(
        x.T.reshape(NCH, 128, B).transpose(1, 0, 2).reshape(128, NCH * B)
    ).astype(_NP_X[DTYPE])

    nc = _get_nc(DTYPE)
    in_maps = []
    for m in range(NCORES):
        # [128, NCH, RC] -> [128, NT, NCH, PW] (r-major pass layout)
        shard = w_t[:, :, m * RC:(m + 1) * RC]
        shard = np.ascontiguousarray(
            shard.reshape(128, NCH, NT, PW).transpose(0, 2, 1, 3)
        )
        in_maps.append({"xt": xt, "w": shard})
    try:
        res = run_bass_kernel_spmd(
            nc, in_maps, core_ids=list(range(NCORES)), trace=_TRACE
        )
    except Exception:
        # transient NRT/device wedges have been observed to clear on retry
        res = run_bass_kernel_spmd(
            nc, in_maps, core_ids=list(range(NCORES)), trace=_TRACE
        )

    y = np.empty((B, R), dtype=np.float32)
    for m in range(NCORES):
        y[:, m * RC:(m + 1) * RC] = res.results[m]["y"]
    if _want_results:
        return y, res
    return y
